# revision 47
# baseline (speedup 1.0000x reference)
"""Trainium2 Bass kernel for nn_DeterministicEgnnPolicy (EGNN message passing).

Strategy (per sharding hint): shard the 1024 independent 32-node graphs
across 8 NeuronCores (128 graphs/core). Dense all-pairs 32x32 edge blocks,
two graph-halves packed into 128 partitions with 64-feature blocks.

All matmuls fp16 (1-pass on PE vs fp32r's 2-pass; the PE sits at the
K=4/8 HAM throttle state = 1.2 GHz for ~95% of the run, so pass count
is the main PE lever). fp16 storage throughout (A/B/pre1/m/h/rad) is
also more accurate than the old bf16 tAB path (HW: 2.8e-3 vs 1.2e-2).
Pipeline: rsc (radial rows) prefetched 4 units ahead; stage distances
S2=+2/S3=+4/S4=+6 with activation consumers deferred one step so each
engine FIFO's head is ready work; four units' S4 (t = Wc2.c1) outputs
land on disjoint partition pairs 2g,2g+1 of one shared PSUM tile via
per-phase lhsT column placement, so the PSUM->SBUF t-extract and the
smat scatter DMAs run once per quad (4x fewer), and P4's single buffer
gains 4 steps of slack. bf16 moving operands measured no faster than
fp16 (both ~0.7 col/cycle at K=4/8) and cost accuracy, so fp16 stays.

Graph indexing on a core: g = gb*4 + gm, gb in [0,32), gm in [0,4).
half = gb//16 (feature partitions 64*half..64*half+63).
node free index (per half): n' = gb_l*128 + gm*32 + i, gb_l = gb%16.
"""

import numpy as np

N_AGENTS = 32
BATCH = 1024
H = 64
L = 4
INV = 16
DEG = float(N_AGENTS - 1)
NCORES = 8
G_CORE = BATCH // NCORES          # 128 graphs per core
NGB = G_CORE // 4                 # 32 gb blocks per core
NGBL = NGB // 2                   # 16 per half
NNODE = NGBL * 128                # 2048 node free dim (per half)
NODES_CORE = G_CORE * N_AGENTS    # 4096

_BUILD_CACHE = {}


# ----------------------------------------------------------------------------
# Host-side packing (pure layout permutation / weight arrangement)
# ----------------------------------------------------------------------------

def _bd(w):
    """64x64 block-diagonal lhsT [128,128] from w [k,64]."""
    k = w.shape[0]
    out = np.zeros((128, 128), np.float32)
    out[0:k, 0:64] = w
    out[64:64 + k, 64:128] = w
    return out


def _bd_rep(wcol):
    """Replicating lhsT: out[64h+f, 64h+f'] = wcol[f] for all f'."""
    out = np.zeros((128, 128), np.float32)
    col = wcol.reshape(64, 1)
    out[0:64, 0:64] = np.repeat(col, 64, axis=1)
    out[64:128, 64:128] = np.repeat(col, 64, axis=1)
    return out


def _wnames():
    names = ["emb"]
    for l in range(L):
        for nm in ("Wi", "Wj", "Wsc", "Wv1", "Wv2",
                   "Wn1t", "Wn1b", "Wn1d", "Wn2"):
            names.append(f"{nm}{l}")
    return names


def _wbnames():
    # bf16 pack: edge-MLP stages whose moving operands are bf16 (full-rate)
    names = []
    for l in range(L):
        for nm in ("We2", "Wc1", "Wc2"):
            names.append(f"{nm}{l}")
    return names


def _bnames():
    names = []
    for l in range(L):
        for nm in ("be1", "be2", "bc1", "bv1", "bn1", "bn2", "bv2", "bc2"):
            names.append(f"{nm}{l}")
    names.append("embb")
    return names


def _pack_weights(inp):
    """Build wpack fp16 [128, NW*128], fpack fp32 [128, 256], biaspack."""
    import ml_dtypes  # noqa: F401
    tiles = {}

    def add(name, arr):
        t = np.zeros((128, 128), np.float32)
        t[:arr.shape[0], :arr.shape[1]] = arr
        tiles[name] = t

    emb = np.zeros((128, 128), np.float32)
    emb[0:INV, 0:64] = inp["emb_W"]
    emb[64:64 + INV, 64:128] = inp["emb_W"]
    add("emb", emb)

    for l in range(L):
        We1 = inp["We1"][l]          # [130, 64]
        add(f"Wi{l}", _bd(We1[0:64]))
        add(f"Wj{l}", _bd(We1[64:128]))
        wsc = np.zeros((4, 128), np.float32)
        wsc[0, 0:64] = We1[128]      # radial, half0
        wsc[1, 0:64] = We1[129]      # edge_attr, half0
        wsc[2, 64:128] = We1[128]
        wsc[3, 64:128] = We1[129]
        add(f"Wsc{l}", wsc)
        add(f"We2{l}", _bd(inp["We2"][l]))
        add(f"Wc1{l}", _bd(inp["Wc1"][l]))
        # Wc2 lhsT, 4 phase variants of 8 cols: variant g puts t(half0) on
        # out partition 2g and t(half1) on 2g+1, so four units' S4 outputs
        # accumulate into disjoint partition pairs of one shared PSUM tile.
        wc2 = np.zeros((128, 128), np.float32)
        for g in range(4):
            wc2[0:64, g * 8 + 2 * g] = inp["Wc2"][l][:, 0]
            wc2[64:128, g * 8 + 2 * g + 1] = inp["Wc2"][l][:, 0]
        add(f"Wc2{l}", wc2)
        add(f"Wv1{l}", _bd(inp["Wv1"][l]))
        add(f"Wv2{l}", _bd_rep(inp["Wv2"][l][:, 0]))
        Wn1 = inp["Wn1"][l]          # [128, 64]
        add(f"Wn1t{l}", _bd(Wn1[0:64]))
        add(f"Wn1b{l}", _bd(Wn1[64:128]))
        add(f"Wn1d{l}", _bd(-Wn1[64:128]))
        add(f"Wn2{l}", _bd(inp["Wn2"][l]))

    wpack = np.concatenate([tiles[n] for n in _wnames()], axis=1).astype(
        np.float16)
    wpackb = np.concatenate([tiles[n] for n in _wbnames()], axis=1).astype(
        np.float16)

    # fp32 pack: identity (transposes) + delta rows, also fp16 delta copy
    fpack = np.zeros((128, 256), np.float32)
    fpack[:, 0:128] = np.eye(128, dtype=np.float32)
    delta = np.zeros((4, 128), np.float32)
    for gm in range(4):
        delta[gm, gm * 32:(gm + 1) * 32] = 1.0
    fpack[0:4, 128:256] = delta

    bias_cols = {}
    for l in range(L):
        for nm in ("be1", "be2", "bc1", "bv1", "bn1", "bn2"):
            bias_cols[f"{nm}{l}"] = np.tile(inp[nm][l].reshape(-1), 2)
        for nm in ("bv2", "bc2"):
            bias_cols[f"{nm}{l}"] = np.full(
                128, float(inp[nm][l].reshape(-1)[0]), np.float32)
    bias_cols["embb"] = np.tile(inp["emb_b"], 2)
    bnames = _bnames()
    biaspack = np.stack([bias_cols[n] for n in bnames], axis=1).astype(
        np.float32)
    return wpack, wpackb, fpack, biaspack


def _arrange_inputs(obs_slice):
    """Per-core obs slice [4096, 20] -> invT fp16 [128, 2048], locvel f32."""
    obs3 = obs_slice.reshape(NGB, 128, 20)          # [gb, (gm,i), col]
    invT = np.zeros((128, NNODE), np.float16)
    inv_half0 = obs3[0:NGBL, :, 0:INV]              # [16, 128, 16]
    inv_half1 = obs3[NGBL:NGB, :, 0:INV]
    invT[0:INV, :] = np.transpose(inv_half0, (2, 0, 1)).reshape(INV, NNODE)
    invT[64:64 + INV, :] = np.transpose(inv_half1, (2, 0, 1)).reshape(
        INV, NNODE)
    locvel = np.ascontiguousarray(
        np.transpose(obs3[:, :, INV:INV + 4], (1, 0, 2)).reshape(128, NGB * 4)
    ).astype(np.float32)
    return invT, locvel


def _unarrange_output(outP):
    """outP [128, 64] -> [4096, 2] (n = gb*128 + p)."""
    return np.ascontiguousarray(
        outP.reshape(128, NGB, 2).transpose(1, 0, 2).reshape(NODES_CORE, 2)
    )


# ----------------------------------------------------------------------------
# Device kernel builder
# ----------------------------------------------------------------------------

def build(scale0, scale1, mean0, mean1):
    import concourse.bacc as bacc
    import concourse.tile as tile
    import concourse.mybir as mybir
    from contextlib import ExitStack

    F32 = mybir.dt.float32
    F16 = mybir.dt.float16
    BF16 = mybir.dt.bfloat16
    AT = mybir.AluOpType
    ACTF = mybir.ActivationFunctionType

    nc = bacc.Bacc("TRN2", target_bir_lowering=False, debug=False)

    invT_d = nc.dram_tensor("invT", [128, NNODE], F16, kind="ExternalInput")
    locvel_d = nc.dram_tensor("locvel", [128, NGB * 4], F32,
                              kind="ExternalInput")
    NW = len(_wnames())
    wpack_d = nc.dram_tensor("wpack", [128, NW * 128], F16,
                             kind="ExternalInput")
    NWB = len(_wbnames())
    wpackb_d = nc.dram_tensor("wpackb", [128, NWB * 128], F16,
                              kind="ExternalInput")
    fpack_d = nc.dram_tensor("fpack", [128, 256], F32, kind="ExternalInput")
    NBIAS = len(_bnames())
    bias_d = nc.dram_tensor("biaspack", [128, NBIAS], F32,
                            kind="ExternalInput")
    out_d = nc.dram_tensor("out", [128, NGB * 2], F32, kind="ExternalOutput")

    widx = {n: i for i, n in enumerate(_wnames())}
    wbidx = {n: i for i, n in enumerate(_wbnames())}
    bidx = {n: i for i, n in enumerate(_bnames())}

    with tile.TileContext(nc) as tc, ExitStack() as ctx:
        st = ctx.enter_context(tc.tile_pool(name="static", bufs=1))
        eA = ctx.enter_context(tc.tile_pool(name="eA", bufs=4))   # m1s
        eM = ctx.enter_context(tc.tile_pool(name="eM", bufs=5))   # m_u
        eQ = ctx.enter_context(tc.tile_pool(name="eQ", bufs=4))   # c1t
        eR = ctx.enter_context(tc.tile_pool(name="eR", bufs=4))   # rsc
        eT = ctx.enter_context(tc.tile_pool(name="eT", bufs=4))   # tAB
        eS = ctx.enter_context(tc.tile_pool(name="eS", bufs=3))   # ssb
        P1 = ctx.enter_context(tc.tile_pool(name="P1", bufs=1, space="PSUM"))
        P2 = ctx.enter_context(tc.tile_pool(name="P2", bufs=1, space="PSUM"))
        P3 = ctx.enter_context(tc.tile_pool(name="P3", bufs=1, space="PSUM"))
        P4 = ctx.enter_context(tc.tile_pool(name="P4", bufs=1, space="PSUM"))

        # ---- static loads ----
        wsb = st.tile([128, NW * 128], F16)
        nc.sync.dma_start(wsb[:], wpack_d.ap())
        wsbb = st.tile([128, NWB * 128], F16)
        nc.sync.dma_start(wsbb[:], wpackb_d.ap())
        fsb = st.tile([128, 256], F32)
        nc.sync.dma_start(fsb[:], fpack_d.ap())
        bsb = st.tile([128, NBIAS], F32)
        nc.sync.dma_start(bsb[:], bias_d.ap())
        locvel = st.tile([128, NGB * 4], F32)
        nc.sync.dma_start(locvel[:], locvel_d.ap())

        def W(name):
            if name[:-1] in ("We2", "Wc1", "Wc2"):
                return wsbb[:, wbidx[name] * 128:(wbidx[name] + 1) * 128]
            return wsb[:, widx[name] * 128:(widx[name] + 1) * 128]

        def Bia(name):
            return bsb[:, bidx[name]:bidx[name] + 1]

        ident = fsb[:, 0:128]
        delta4f = fsb[0:4, 128:256]

        # ---- persistent state ----
        hA = st.tile([128, NNODE], F16)
        hB = st.tile([128, NNODE], F16)
        # hB doubles as the invT staging buffer (consumed by the embedding
        # before layer 0's h-update writes h_next into it)
        invT = hB
        nc.sync.dma_start(invT[:], invT_d.ap())
        magg = st.tile([128, NNODE], F16)
        mdiag = st.tile([128, NNODE], F16)
        smat = st.tile([128, 1024], F32)
        rad = st.tile([128, 1024], F16)
        ea = st.tile([128, 1024], F16)
        dx = st.tile([128, 1024], F32)
        dy = st.tile([128, 1024], F32)
        locx = st.tile([128, NGB], F32)
        locy = st.tile([128, NGB], F32)
        velx = st.tile([128, NGB], F32)
        vely = st.tile([128, NGB], F32)
        phiP = st.tile([128, NGB], F32)
        hv1 = st.tile([128, 1024], F16)
        phirep = st.tile([128, NNODE], F32)
        lxT = st.tile([32, 128], F16)
        lyT = st.tile([32, 128], F16)
        delta4 = st.tile([4, 128], F16)
        T4x = st.tile([4, 1024], F16)
        T4y = st.tile([4, 1024], F16)
        tm = st.tile([128, 1024], F32)
        outP = st.tile([128, NGB * 2], F32)
        A8 = st.tile([128, NNODE], F16)
        B8 = st.tile([128, NNODE], F16)

        nc.vector.tensor_copy(delta4[:], delta4f[:])
        lv = locvel[:].rearrange("p (gb c) -> p gb c", c=4)
        nc.vector.tensor_copy(locx[:], lv[:, :, 0])
        nc.vector.tensor_copy(locy[:], lv[:, :, 1])
        nc.vector.tensor_copy(velx[:], lv[:, :, 2])
        nc.vector.tensor_copy(vely[:], lv[:, :, 3])

        def heat(lhsT_ap, rhs_ap, n=14):
            hp = P4.tile([128, 1024], F32, tag="stage")
            for _ in range(n):
                nc.tensor.matmul(hp[:, 0:512], lhsT_ap, rhs_ap,
                                 start=True, stop=True)

        # ---- embedding: h0 = inv @ emb_W + emb_b ----
        heat(W("emb"), invT[:, 0:512])
        for u in range(NNODE // 1024):
            pse = (P1 if u == 0 else P2).tile([128, 1024], F32, tag="stage")
            for k in range(2):
                sl = slice(u * 1024 + k * 512, u * 1024 + (k + 1) * 512)
                nc.tensor.matmul(pse[:, k * 512:(k + 1) * 512], W("emb"),
                                 invT[:, sl], start=True, stop=True)
            nc.vector.tensor_scalar_add(hA[:, u * 1024:(u + 1) * 1024],
                                        pse[:], Bia("embb"))

        def radial_part(first):
            """Compute lxT/lyT, T4s, dx, dy, rad from current locx/locy."""
            for (lP, lT) in ((locx, lxT), (locy, lyT)):
                pst = P4.tile([128, 1024], F32, tag="stage")
                nc.tensor.transpose(pst[0:32, 0:128], lP[:], ident)
                nc.vector.tensor_copy(lT[:], pst[0:32, 0:128])
            for (lT, T4) in ((lxT, T4x), (lyT, T4y)):
                for gm in range(4):
                    nc.sync.dma_start(
                        T4[gm:gm + 1, :].rearrange("p (gb j) -> p gb j", j=32),
                        lT[:, gm * 32:(gm + 1) * 32])
            for (T4, lP, dT) in ((T4x, locx, dx), (T4y, locy, dy)):
                pss = P1.tile([128, 1024], F32, tag="stage")
                for k in range(2):
                    nc.tensor.matmul(pss[:, k * 512:(k + 1) * 512], delta4[:],
                                     T4[:, k * 512:(k + 1) * 512],
                                     start=True, stop=True)
                bc = lP[:].unsqueeze(2).broadcast_to([128, NGB, 32])
                nc.vector.tensor_tensor(
                    dT[:].rearrange("p (gb j) -> p gb j", j=32), bc,
                    pss[:].rearrange("p (gb j) -> p gb j", j=32),
                    op=AT.subtract)
            t2 = st.tile([128, 1024], F32, tag="mx_t2")
            nc.vector.tensor_tensor(t2[:], dx[:], dx[:], op=AT.mult)
            with nc.allow_low_precision(reason="fp16 rad"):
                nc.vector.tensor_tensor(rad[:], dy[:], dy[:], op=AT.mult)
                nc.vector.tensor_tensor(rad[:], rad[:], t2[:], op=AT.add)
            if first:
                nc.vector.tensor_copy(ea[:], rad[:])

        radial_part(first=True)

        # per-layer edge unit emitters ---------------------------------------
        # edge stage 1 off the PE: A = Wi.h, B = Wj.h are per-node; per-edge
        # pre-act = A_i + B_j (GpSimd fp16 broadcast add) + wr*rad + we*ea
        # (4-row Wsc matmul) summed by one DVE op.
        def emit_tAB(l, i):
            gb_l, gmp, u = i >> 2, (i >> 1) & 1, i & 1
            nb = gb_l * 128 + gmp * 64 + u * 32
            tAB = eT.tile([128, 1024], F16, tag="tAB")
            nc.gpsimd.tensor_tensor(
                tAB[:].rearrange("p (i j) -> p i j", j=32),
                A8[:, nb:nb + 32].unsqueeze(2).broadcast_to([128, 32, 32]),
                B8[:, nb:nb + 32].unsqueeze(1).broadcast_to([128, 32, 32]),
                op=AT.add)
            return tAB

        rsc_q = {}

        def emit_rsc(pair):
            """Prefetch the radial/edge_attr rows for unit pair `pair`."""
            i = pair * 2
            gb_l, gmp = i >> 2, (i >> 1) & 1
            p0 = gmp * 64
            rsc = eR.tile([4, 2048], F16, tag="rsc")
            for (row, src) in ((0, rad), (1, ea)):
                nc.sync.dma_start(
                    rsc[row:row + 1, :].rearrange(
                        "p (a b c) -> p a b c", a=2, b=32, c=32),
                    src[p0:p0 + 64, gb_l * 32:(gb_l + 1) * 32])
                nc.sync.dma_start(
                    rsc[row + 2:row + 3, :].rearrange(
                        "p (a b c) -> p a b c", a=2, b=32, c=32),
                    src[p0:p0 + 64, (gb_l + 16) * 32:(gb_l + 17) * 32])
            rsc_q[pair] = rsc

        def emit_S1(l, h, i, tAB):
            gb_l, gmp, u = i >> 2, (i >> 1) & 1, i & 1
            rsc = rsc_q[i >> 1]
            if u == 1:
                del rsc_q[i >> 1]
            ps1 = P1.tile([128, 1024], F32, tag="stage")
            for k in range(2):
                ksl = slice(u * 1024 + k * 512, u * 1024 + (k + 1) * 512)
                nc.tensor.matmul(ps1[:, k * 512:(k + 1) * 512],
                                 W(f"Wsc{l}")[0:4, :], rsc[:, ksl],
                                 start=True, stop=True)
            pre1 = eT.tile([128, 1024], F16, tag="pre1")
            with nc.allow_low_precision(reason="fp16 pre1"):
                nc.vector.tensor_tensor(pre1[:], tAB[:], ps1[:], op=AT.add)
            return pre1

        def emit_A1(l, st_u):
            m1s = eA.tile([128, 1024], F16, tag="m1s")
            nc.scalar.activation(m1s[:], st_u["pre1"][:], ACTF.Silu,
                                 bias=Bia(f"be1{l}"))
            st_u["m1s"] = m1s

        def emit_S2(l, st_u):
            ps2 = P2.tile([128, 1024], F32, tag="stage")
            m1s = st_u["m1s"]
            for k in range(2):
                ksl = slice(k * 512, (k + 1) * 512)
                nc.tensor.matmul(ps2[:, ksl], W(f"We2{l}"), m1s[:, ksl],
                                 start=True, stop=True)
            st_u["ps2"] = ps2

        def emit_A2(l, st_u):
            m_u = eM.tile([128, 1024], F16, tag="m_u")
            nc.scalar.activation(m_u[:], st_u["ps2"][:], ACTF.Silu,
                                 bias=Bia(f"be2{l}"))
            st_u["m_u"] = m_u

        def emit_red(l, st_u):
            nb = st_u["nb"]
            m_u = st_u["m_u"]
            with nc.allow_low_precision(reason="fp16 magg"):
                nc.vector.tensor_reduce(
                    magg[:, nb:nb + 32],
                    m_u[:].rearrange("p (i j) -> p i j", j=32),
                    axis=mybir.AxisListType.X, op=AT.add)
            nc.gpsimd.tensor_copy(mdiag[:, nb:nb + 32], m_u[:, 0:1024:33])

        def emit_S3(l, st_u):
            ps3 = P3.tile([128, 1024], F32, tag="stage")
            m_u = st_u["m_u"]
            for k in range(2):
                ksl = slice(k * 512, (k + 1) * 512)
                nc.tensor.matmul(ps3[:, ksl], W(f"Wc1{l}"), m_u[:, ksl],
                                 start=True, stop=True)
            st_u["ps3"] = ps3

        def emit_sq(l, st_u):
            c1t = eQ.tile([128, 1024], F16, tag="c1t")
            nc.scalar.activation(c1t[:], st_u["ps3"][:], ACTF.Silu,
                                 bias=Bia(f"bc1{l}"))
            st_u["c1t"] = c1t

        def emit_S4(l, st_u):
            g = st_u["i"] & 3
            if g == 0:
                emit_S4.ps4 = P4.tile([128, 1024], F32, tag="stage")
            ps4 = emit_S4.ps4
            c1t = st_u["c1t"]
            wsl = W(f"Wc2{l}")[:, g * 8:(g + 1) * 8]
            for k in range(2):
                ksl = slice(k * 512, (k + 1) * 512)
                nc.tensor.matmul(ps4[0:8, ksl], wsl, c1t[:, ksl],
                                 start=(g == 0), stop=(g == 3),
                                 skip_group_check=True)
            st_u["ps4"] = ps4

        def emit_out(l, st_u):
            # called on the quad's last unit: ps4 partitions 0..7 hold t for
            # 4 units x 2 halves. One extract (+bc2) and two DMAs per quad.
            i = st_u["i"]
            gb_l = i >> 2
            ps4 = st_u["ps4"]
            ssb = eS.tile([8, 1024], F32, tag="ssb")
            bc2ap = bsb[0:8, bidx[f"bc2{l}"]:bidx[f"bc2{l}"] + 1]
            nc.vector.tensor_scalar_add(ssb[:], ps4[0:8, :], bc2ap)
            nc.sync.dma_start(
                smat[:, gb_l * 32:(gb_l + 1) * 32],
                ssb[0:8:2, :].rearrange("p (i j) -> p i j", j=32))
            nc.sync.dma_start(
                smat[:, (gb_l + 16) * 32:(gb_l + 17) * 32],
                ssb[1:8:2, :].rearrange("p (i j) -> p i j", j=32))

        def emit_hupd(l, h, h_next, u):
            sl = slice(u * 1024, (u + 1) * 1024)
            psh = P3.tile([128, 1024], F32, tag="stage")
            for k in range(2):
                ksl = slice(u * 1024 + k * 512, u * 1024 + (k + 1) * 512)
                osl = slice(k * 512, (k + 1) * 512)
                nc.tensor.matmul(psh[:, osl], W(f"Wn1t{l}"), h[:, ksl],
                                 start=True, stop=False)
                nc.tensor.matmul(psh[:, osl], W(f"Wn1b{l}"), magg[:, ksl],
                                 start=False, stop=False)
                nc.tensor.matmul(psh[:, osl], W(f"Wn1d{l}"), mdiag[:, ksl],
                                 start=False, stop=True)
            hn1 = eA.tile([128, 1024], F16, tag="hn1")
            nc.scalar.activation(hn1[:], psh[:], ACTF.Silu,
                                 bias=Bia(f"bn1{l}"))
            # P3 (not P4): P4 generations now span 4 pipeline steps
            psh2 = P3.tile([128, 1024], F32, tag="stage")
            for k in range(2):
                osl = slice(k * 512, (k + 1) * 512)
                nc.tensor.matmul(psh2[:, osl], W(f"Wn2{l}"), hn1[:, osl],
                                 start=True, stop=True)
            with nc.allow_low_precision(reason="fp16 h"):
                nc.vector.scalar_tensor_tensor(
                    h_next[:, sl], psh2[:], Bia(f"bn2{l}"), h[:, sl],
                    op0=AT.add, op1=AT.add)

        def node_AB(l, h):
            """Per-node A = Wi.h, B = Wj.h and phi -> phirep for layer l."""
            for (wn, dst) in ((f"Wi{l}", A8), (f"Wj{l}", B8)):
                for u in range(NNODE // 1024):
                    psab = P1.tile([128, 1024], F32, tag="stage")
                    for k in range(2):
                        ksl = slice(u * 1024 + k * 512,
                                    u * 1024 + (k + 1) * 512)
                        nc.tensor.matmul(psab[:, k * 512:(k + 1) * 512],
                                         W(wn), h[:, ksl],
                                         start=True, stop=True)
                    with nc.allow_low_precision(reason="fp16 AB"):
                        nc.vector.tensor_copy(
                            dst[:, u * 1024:(u + 1) * 1024], psab[:])
            for u in range(NNODE // 1024):
                sl = slice(u * 1024, (u + 1) * 1024)
                psv = P1.tile([128, 1024], F32, tag="stage")
                for k in range(2):
                    ksl = slice(u * 1024 + k * 512, u * 1024 + (k + 1) * 512)
                    nc.tensor.matmul(psv[:, k * 512:(k + 1) * 512],
                                     W(f"Wv1{l}"), h[:, ksl],
                                     start=True, stop=True)
                nc.scalar.activation(hv1[:], psv[:], ACTF.Silu,
                                     bias=Bia(f"bv1{l}"))
                psv2 = P2.tile([128, 1024], F32, tag="stage")
                for k in range(2):
                    nc.tensor.matmul(psv2[:, k * 512:(k + 1) * 512],
                                     W(f"Wv2{l}"),
                                     hv1[:, k * 512:(k + 1) * 512],
                                     start=True, stop=True)
                nc.vector.tensor_scalar_add(phirep[:, sl], psv2[:],
                                            Bia(f"bv2{l}"))

        def phi_extract():
            # batch 8 transposed blocks per PSUM tile, then 2 strided copies
            for g in range(NGBL // 8):
                pst = P3.tile([128, 1024], F32, tag="stage")
                for b in range(8):
                    c = g * 8 + b
                    nc.tensor.transpose(pst[:, b * 128:(b + 1) * 128],
                                        phirep[:, c * 128:(c + 1) * 128],
                                        ident)
                nc.vector.tensor_copy(phiP[:, g * 8:(g + 1) * 8],
                                      pst[:, 0:1024:128])
                nc.vector.tensor_copy(phiP[:, NGBL + g * 8:NGBL + (g + 1) * 8],
                                      pst[:, 64:1024:128])

        def tm_chain():
            nc.scalar.activation(tm[:], rad[:], ACTF.Sqrt)
            nc.vector.tensor_scalar_add(tm[:], tm[:], 1.0)
            nc.vector.reciprocal(tm[:], tm[:])

        for l in range(L):
            h = hA if l % 2 == 0 else hB
            h_next = hB if l % 2 == 0 else hA

            if l == 0:
                tm_chain()
                node_AB(0, h)
                phi_extract()

            # ---- software-pipelined edge units ----
            NU = 64
            stq = {}

            # deep pipeline: S2 two steps behind S1, S3 four, S4 six, so
            # every stage's input is >=2 steps old when the PE reaches it
            D2, D3, D4 = 2, 4, 6

            def pipe_step(i):
                # rsc DMAs prefetched 4 units ahead so S1 never waits on them
                if i == 0:
                    for p in (0, 1, 2):
                        emit_rsc(p)
                j = i + 5
                if j < NU and (j & 1) == 0:
                    emit_rsc(j >> 1)
                if i == 0:
                    stq[0] = {"i": 0, "nb": 0, "tAB": emit_tAB(l, 0)}
                if i + 1 < NU:
                    stq[i + 1] = {"i": i + 1,
                                  "nb": ((i + 1) >> 2) * 128
                                  + ((i + 1) & 3) * 32,
                                  "tAB": emit_tAB(l, i + 1)}
                # activation/reduce consumers FIRST (their PSUM inputs were
                # written in prior steps, so each engine's FIFO head is
                # ready work and PE outputs of THIS step gate nothing ahead
                # of it), then the PE stages.
                if 0 <= i - D2 - 1 < NU:
                    emit_A2(l, stq[i - D2 - 1])
                if 0 <= i - D3 - 1 < NU:
                    emit_sq(l, stq[i - D3 - 1])
                if 0 <= i - 1 < NU:
                    emit_A1(l, stq[i - 1])
                if 0 <= i - D2 - 1 < NU:
                    emit_red(l, stq[i - D2 - 1])
                if i < NU:
                    stq[i]["pre1"] = emit_S1(l, h, i, stq[i]["tAB"])
                if 0 <= i - D2 < NU:
                    emit_S2(l, stq[i - D2])
                if 0 <= i - D4 < NU:
                    emit_S4(l, stq[i - D4])
                    if ((i - D4) & 3) == 3:
                        emit_out(l, stq[i - D4])
                if 0 <= i - D3 < NU:
                    emit_S3(l, stq[i - D3])
                if 0 <= i - D4 < NU:
                    del stq[i - D4]

            for i in range(NU + D4 + 1):
                pipe_step(i)
            # both h-update halves run post-loop: injecting one mid-pipe
            # stalls the S3 stream ~4-5us on the P3 ring (psh/psh2 readers)
            emit_hupd(l, h, h_next, 0)
            emit_hupd(l, h, h_next, 1)
            # next layer's node-phase PE work overlaps this layer's
            # (vector/scalar-only) matrix phase
            if l < L - 1:
                node_AB(l + 1, h_next)

            # ---- matrix phase: um, agg, vel/loc update; then radial(l+1) --
            um = st.tile([128, 1024], F32, tag="mx_um")
            nc.vector.tensor_tensor(um[:], smat[:], tm[:], op=AT.mult)
            for (dT, agg_out) in ((dx, "ax"), (dy, "ay")):
                w_ = st.tile([128, 1024], F32, tag="mx_w2")
                nc.vector.tensor_tensor(w_[:], um[:], dT[:], op=AT.mult)
                ag = st.tile([128, NGB], F32, tag="mx_" + agg_out)
                nc.vector.tensor_reduce(
                    ag[:], w_[:].rearrange("p (gb j) -> p gb j", j=32),
                    axis=mybir.AxisListType.X, op=AT.add)
                vP = velx if agg_out == "ax" else vely
                tmp = st.tile([128, NGB], F32, tag="mx_tmp")
                nc.vector.tensor_tensor(tmp[:], phiP[:], vP[:], op=AT.mult)
                nc.vector.scalar_tensor_tensor(vP[:], ag[:], 1.0 / DEG,
                                               tmp[:], op0=AT.mult,
                                               op1=AT.add)
            nc.vector.tensor_tensor(locx[:], locx[:], velx[:], op=AT.add)
            nc.vector.tensor_tensor(locy[:], locy[:], vely[:], op=AT.add)
            if l < L - 1:
                radial_part(first=False)
                tm_chain()
                phi_extract()

        # ---- output: outP interleaved (gb, c) ----
        ov = outP[:].rearrange("p (gb c) -> p gb c", c=2)
        nc.vector.tensor_scalar(ov[:, :, 0], velx[:], scale0, mean0,
                                op0=AT.mult, op1=AT.add)
        nc.vector.tensor_scalar(ov[:, :, 1], vely[:], scale1, mean1,
                                op0=AT.mult, op1=AT.add)
        nc.sync.dma_start(out_d.ap(), outP[:])

    nc.compile()
    return nc


# ----------------------------------------------------------------------------
# Entry point
# ----------------------------------------------------------------------------

def kernel(**inputs):
    import concourse.mybir  # noqa: F401  (ensure env importable)
    from concourse.bass_utils import run_bass_kernel_spmd

    inp = {k: np.asarray(v) for k, v in inputs.items()}
    obs = inp["obs"].astype(np.float32)
    scale = np.asarray(inp["scale"], np.float32)
    mean = np.asarray(inp["mean"], np.float32)

    key = (float(scale[0]), float(scale[1]), float(mean[0]), float(mean[1]))
    if key not in _BUILD_CACHE:
        _BUILD_CACHE[key] = build(*key)
    nc = _BUILD_CACHE[key]

    wpack, wpackb, fpack, biaspack = _pack_weights(inp)
    in_maps = []
    for c in range(NCORES):
        invT, locvel = _arrange_inputs(obs[c * NODES_CORE:(c + 1) * NODES_CORE])
        in_maps.append({"invT": invT, "locvel": locvel, "wpack": wpack,
                       "wpackb": wpackb, "fpack": fpack,
                        "biaspack": biaspack})
    res = run_bass_kernel_spmd(nc, in_maps, list(range(NCORES)))
    outs = [_unarrange_output(res.results[c]["out"]) for c in range(NCORES)]
    return np.concatenate(outs, axis=0)


# revision 49
# speedup vs baseline: 1.0069x; 1.0069x over previous
"""Trainium2 Bass kernel for nn_DeterministicEgnnPolicy (EGNN message passing).

Strategy (per sharding hint): shard the 1024 independent 32-node graphs
across 8 NeuronCores (128 graphs/core). Dense all-pairs 32x32 edge blocks,
two graph-halves packed into 128 partitions with 64-feature blocks.

All matmuls fp16 (1-pass on PE vs fp32r's 2-pass; the PE sits at the
K=4/8 HAM throttle state = 1.2 GHz for ~95% of the run, so pass count
is the main PE lever). fp16 storage throughout (A/B/pre1/m/h/rad) is
also more accurate than the old bf16 tAB path (HW: 2.8e-3 vs 1.2e-2).
Pipeline: rsc (radial rows) prefetched 4 units ahead; stage distances
S2=+2/S3=+4/S4=+6 with activation consumers deferred one step so each
engine FIFO's head is ready work; four units' S4 (t = Wc2.c1) outputs
land on disjoint partition pairs 2g,2g+1 of one shared PSUM tile via
per-phase lhsT column placement, so the PSUM->SBUF t-extract and the
smat scatter DMAs run once per quad (4x fewer), and P4's single buffer
gains 4 steps of slack. bf16 moving operands measured no faster than
fp16 (both ~0.7 col/cycle at K=4/8) and cost accuracy, so fp16 stays.

Graph indexing on a core: g = gb*4 + gm, gb in [0,32), gm in [0,4).
half = gb//16 (feature partitions 64*half..64*half+63).
node free index (per half): n' = gb_l*128 + gm*32 + i, gb_l = gb%16.
"""

import numpy as np

N_AGENTS = 32
BATCH = 1024
H = 64
L = 4
INV = 16
DEG = float(N_AGENTS - 1)
NCORES = 8
G_CORE = BATCH // NCORES          # 128 graphs per core
NGB = G_CORE // 4                 # 32 gb blocks per core
NGBL = NGB // 2                   # 16 per half
NNODE = NGBL * 128                # 2048 node free dim (per half)
NODES_CORE = G_CORE * N_AGENTS    # 4096

_BUILD_CACHE = {}


# ----------------------------------------------------------------------------
# Host-side packing (pure layout permutation / weight arrangement)
# ----------------------------------------------------------------------------

def _bd(w):
    """64x64 block-diagonal lhsT [128,128] from w [k,64]."""
    k = w.shape[0]
    out = np.zeros((128, 128), np.float32)
    out[0:k, 0:64] = w
    out[64:64 + k, 64:128] = w
    return out


def _bd_rep(wcol):
    """Replicating lhsT: out[64h+f, 64h+f'] = wcol[f] for all f'."""
    out = np.zeros((128, 128), np.float32)
    col = wcol.reshape(64, 1)
    out[0:64, 0:64] = np.repeat(col, 64, axis=1)
    out[64:128, 64:128] = np.repeat(col, 64, axis=1)
    return out


def _wnames():
    names = ["emb"]
    for l in range(L):
        for nm in ("Wi", "Wj", "Wsc", "Wv1", "Wv2",
                   "Wn1t", "Wn1b", "Wn1d", "Wn2"):
            names.append(f"{nm}{l}")
    return names


def _wbnames():
    # bf16 pack: edge-MLP stages whose moving operands are bf16 (full-rate)
    names = []
    for l in range(L):
        for nm in ("We2", "Wc1", "Wc2"):
            names.append(f"{nm}{l}")
    return names


def _bnames():
    names = []
    for l in range(L):
        for nm in ("be1", "be2", "bc1", "bv1", "bn1", "bn2", "bv2", "bc2"):
            names.append(f"{nm}{l}")
    names.append("embb")
    return names


def _pack_weights(inp):
    """Build wpack fp16 [128, NW*128], fpack fp32 [128, 256], biaspack."""
    import ml_dtypes  # noqa: F401
    tiles = {}

    def add(name, arr):
        t = np.zeros((128, 128), np.float32)
        t[:arr.shape[0], :arr.shape[1]] = arr
        tiles[name] = t

    emb = np.zeros((128, 128), np.float32)
    emb[0:INV, 0:64] = inp["emb_W"]
    emb[64:64 + INV, 64:128] = inp["emb_W"]
    add("emb", emb)

    for l in range(L):
        We1 = inp["We1"][l]          # [130, 64]
        add(f"Wi{l}", _bd(We1[0:64]))
        add(f"Wj{l}", _bd(We1[64:128]))
        wsc = np.zeros((4, 128), np.float32)
        wsc[0, 0:64] = We1[128]      # radial, half0
        wsc[1, 0:64] = We1[129]      # edge_attr, half0
        wsc[2, 64:128] = We1[128]
        wsc[3, 64:128] = We1[129]
        add(f"Wsc{l}", wsc)
        add(f"We2{l}", _bd(inp["We2"][l]))
        add(f"Wc1{l}", _bd(inp["Wc1"][l]))
        # Wc2 lhsT, 4 phase variants of 8 cols: variant g puts t(half0) on
        # out partition 2g and t(half1) on 2g+1, so four units' S4 outputs
        # accumulate into disjoint partition pairs of one shared PSUM tile.
        wc2 = np.zeros((128, 128), np.float32)
        for g in range(4):
            wc2[0:64, g * 8 + 2 * g] = inp["Wc2"][l][:, 0]
            wc2[64:128, g * 8 + 2 * g + 1] = inp["Wc2"][l][:, 0]
        add(f"Wc2{l}", wc2)
        add(f"Wv1{l}", _bd(inp["Wv1"][l]))
        add(f"Wv2{l}", _bd_rep(inp["Wv2"][l][:, 0]))
        Wn1 = inp["Wn1"][l]          # [128, 64]
        add(f"Wn1t{l}", _bd(Wn1[0:64]))
        add(f"Wn1b{l}", _bd(Wn1[64:128]))
        add(f"Wn1d{l}", _bd(-Wn1[64:128]))
        add(f"Wn2{l}", _bd(inp["Wn2"][l]))

    wpack = np.concatenate([tiles[n] for n in _wnames()], axis=1).astype(
        np.float16)
    wpackb = np.concatenate([tiles[n] for n in _wbnames()], axis=1).astype(
        np.float16)

    # fp32 pack: identity (transposes) + delta rows, also fp16 delta copy
    fpack = np.zeros((128, 256), np.float32)
    fpack[:, 0:128] = np.eye(128, dtype=np.float32)
    delta = np.zeros((4, 128), np.float32)
    for gm in range(4):
        delta[gm, gm * 32:(gm + 1) * 32] = 1.0
    fpack[0:4, 128:256] = delta

    bias_cols = {}
    for l in range(L):
        for nm in ("be1", "be2", "bc1", "bv1", "bn1", "bn2"):
            bias_cols[f"{nm}{l}"] = np.tile(inp[nm][l].reshape(-1), 2)
        for nm in ("bv2", "bc2"):
            bias_cols[f"{nm}{l}"] = np.full(
                128, float(inp[nm][l].reshape(-1)[0]), np.float32)
    bias_cols["embb"] = np.tile(inp["emb_b"], 2)
    bnames = _bnames()
    biaspack = np.stack([bias_cols[n] for n in bnames], axis=1).astype(
        np.float32)
    return wpack, wpackb, fpack, biaspack


def _arrange_inputs(obs_slice):
    """Per-core obs slice [4096, 20] -> invT fp16 [128, 2048], locvel f32."""
    obs3 = obs_slice.reshape(NGB, 128, 20)          # [gb, (gm,i), col]
    invT = np.zeros((128, NNODE), np.float16)
    inv_half0 = obs3[0:NGBL, :, 0:INV]              # [16, 128, 16]
    inv_half1 = obs3[NGBL:NGB, :, 0:INV]
    invT[0:INV, :] = np.transpose(inv_half0, (2, 0, 1)).reshape(INV, NNODE)
    invT[64:64 + INV, :] = np.transpose(inv_half1, (2, 0, 1)).reshape(
        INV, NNODE)
    locvel = np.ascontiguousarray(
        np.transpose(obs3[:, :, INV:INV + 4], (1, 0, 2)).reshape(128, NGB * 4)
    ).astype(np.float32)
    return invT, locvel


def _unarrange_output(outP):
    """outP [128, 64] -> [4096, 2] (n = gb*128 + p)."""
    return np.ascontiguousarray(
        outP.reshape(128, NGB, 2).transpose(1, 0, 2).reshape(NODES_CORE, 2)
    )


# ----------------------------------------------------------------------------
# Device kernel builder
# ----------------------------------------------------------------------------

def build(scale0, scale1, mean0, mean1):
    import concourse.bacc as bacc
    import concourse.tile as tile
    import concourse.mybir as mybir
    from contextlib import ExitStack

    F32 = mybir.dt.float32
    F16 = mybir.dt.float16
    BF16 = mybir.dt.bfloat16
    AT = mybir.AluOpType
    ACTF = mybir.ActivationFunctionType

    nc = bacc.Bacc("TRN2", target_bir_lowering=False, debug=False)

    invT_d = nc.dram_tensor("invT", [128, NNODE], F16, kind="ExternalInput")
    locvel_d = nc.dram_tensor("locvel", [128, NGB * 4], F32,
                              kind="ExternalInput")
    NW = len(_wnames())
    wpack_d = nc.dram_tensor("wpack", [128, NW * 128], F16,
                             kind="ExternalInput")
    NWB = len(_wbnames())
    wpackb_d = nc.dram_tensor("wpackb", [128, NWB * 128], F16,
                              kind="ExternalInput")
    fpack_d = nc.dram_tensor("fpack", [128, 256], F32, kind="ExternalInput")
    NBIAS = len(_bnames())
    bias_d = nc.dram_tensor("biaspack", [128, NBIAS], F32,
                            kind="ExternalInput")
    out_d = nc.dram_tensor("out", [128, NGB * 2], F32, kind="ExternalOutput")

    widx = {n: i for i, n in enumerate(_wnames())}
    wbidx = {n: i for i, n in enumerate(_wbnames())}
    bidx = {n: i for i, n in enumerate(_bnames())}

    with tile.TileContext(nc) as tc, ExitStack() as ctx:
        st = ctx.enter_context(tc.tile_pool(name="static", bufs=1))
        eA = ctx.enter_context(tc.tile_pool(name="eA", bufs=4))   # m1s
        eM = ctx.enter_context(tc.tile_pool(name="eM", bufs=5))   # m_u
        eQ = ctx.enter_context(tc.tile_pool(name="eQ", bufs=4))   # c1t
        eR = ctx.enter_context(tc.tile_pool(name="eR", bufs=4))   # rsc
        eT = ctx.enter_context(tc.tile_pool(name="eT", bufs=4))   # tAB
        eS = ctx.enter_context(tc.tile_pool(name="eS", bufs=3))   # ssb
        P1 = ctx.enter_context(tc.tile_pool(name="P1", bufs=1, space="PSUM"))
        P2 = ctx.enter_context(tc.tile_pool(name="P2", bufs=1, space="PSUM"))
        P3 = ctx.enter_context(tc.tile_pool(name="P3", bufs=1, space="PSUM"))
        P4 = ctx.enter_context(tc.tile_pool(name="P4", bufs=1, space="PSUM"))

        # ---- static loads ----
        wsb = st.tile([128, NW * 128], F16)
        nc.sync.dma_start(wsb[:], wpack_d.ap())
        wsbb = st.tile([128, NWB * 128], F16)
        nc.sync.dma_start(wsbb[:], wpackb_d.ap())
        fsb = st.tile([128, 256], F32)
        nc.sync.dma_start(fsb[:], fpack_d.ap())
        bsb = st.tile([128, NBIAS], F32)
        nc.sync.dma_start(bsb[:], bias_d.ap())
        locvel = st.tile([128, NGB * 4], F32)
        nc.sync.dma_start(locvel[:], locvel_d.ap())

        def W(name):
            if name[:-1] in ("We2", "Wc1", "Wc2"):
                return wsbb[:, wbidx[name] * 128:(wbidx[name] + 1) * 128]
            return wsb[:, widx[name] * 128:(widx[name] + 1) * 128]

        def Bia(name):
            return bsb[:, bidx[name]:bidx[name] + 1]

        ident = fsb[:, 0:128]
        delta4f = fsb[0:4, 128:256]

        # ---- persistent state ----
        hA = st.tile([128, NNODE], F16)
        hB = st.tile([128, NNODE], F16)
        # hB doubles as the invT staging buffer (consumed by the embedding
        # before layer 0's h-update writes h_next into it)
        invT = hB
        nc.sync.dma_start(invT[:], invT_d.ap())
        magg = st.tile([128, NNODE], F16)
        mdiag = st.tile([128, NNODE], F16)
        smat = st.tile([128, 1024], F32)
        rad = st.tile([128, 1024], F16)
        ea = st.tile([128, 1024], F16)
        dx = st.tile([128, 1024], F32)
        dy = st.tile([128, 1024], F32)
        locx = st.tile([128, NGB], F32)
        locy = st.tile([128, NGB], F32)
        velx = st.tile([128, NGB], F32)
        vely = st.tile([128, NGB], F32)
        phiP = st.tile([128, NGB], F32)
        hv1 = st.tile([128, 1024], F16)
        phirep = st.tile([128, NNODE], F32)
        lxT = st.tile([32, 128], F16)
        lyT = st.tile([32, 128], F16)
        delta4 = st.tile([4, 128], F16)
        T4x = st.tile([4, 1024], F16)
        T4y = st.tile([4, 1024], F16)
        tm = st.tile([128, 1024], F32)
        outP = st.tile([128, NGB * 2], F32)
        A8 = st.tile([128, NNODE], F16)
        B8 = st.tile([128, NNODE], F16)

        nc.vector.tensor_copy(delta4[:], delta4f[:])
        lv = locvel[:].rearrange("p (gb c) -> p gb c", c=4)
        nc.vector.tensor_copy(locx[:], lv[:, :, 0])
        nc.vector.tensor_copy(locy[:], lv[:, :, 1])
        nc.vector.tensor_copy(velx[:], lv[:, :, 2])
        nc.vector.tensor_copy(vely[:], lv[:, :, 3])

        def heat(lhsT_ap, rhs_ap, n=14):
            hp = P4.tile([128, 1024], F32, tag="stage")
            for _ in range(n):
                nc.tensor.matmul(hp[:, 0:512], lhsT_ap, rhs_ap,
                                 start=True, stop=True)

        # ---- embedding: h0 = inv @ emb_W + emb_b ----
        heat(W("emb"), invT[:, 0:512])
        for u in range(NNODE // 1024):
            pse = (P1 if u == 0 else P2).tile([128, 1024], F32, tag="stage")
            for k in range(2):
                sl = slice(u * 1024 + k * 512, u * 1024 + (k + 1) * 512)
                nc.tensor.matmul(pse[:, k * 512:(k + 1) * 512], W("emb"),
                                 invT[:, sl], start=True, stop=True)
            nc.vector.tensor_scalar_add(hA[:, u * 1024:(u + 1) * 1024],
                                        pse[:], Bia("embb"))

        def radial_part(first):
            """Compute lxT/lyT, T4s, dx, dy, rad from current locx/locy."""
            for (lP, lT) in ((locx, lxT), (locy, lyT)):
                pst = P4.tile([128, 1024], F32, tag="stage")
                nc.tensor.transpose(pst[0:32, 0:128], lP[:], ident)
                nc.vector.tensor_copy(lT[:], pst[0:32, 0:128])
            for (lT, T4) in ((lxT, T4x), (lyT, T4y)):
                for gm in range(4):
                    nc.sync.dma_start(
                        T4[gm:gm + 1, :].rearrange("p (gb j) -> p gb j", j=32),
                        lT[:, gm * 32:(gm + 1) * 32])
            for (T4, lP, dT) in ((T4x, locx, dx), (T4y, locy, dy)):
                pss = P1.tile([128, 1024], F32, tag="stage")
                for k in range(2):
                    nc.tensor.matmul(pss[:, k * 512:(k + 1) * 512], delta4[:],
                                     T4[:, k * 512:(k + 1) * 512],
                                     start=True, stop=True)
                bc = lP[:].unsqueeze(2).broadcast_to([128, NGB, 32])
                nc.vector.tensor_tensor(
                    dT[:].rearrange("p (gb j) -> p gb j", j=32), bc,
                    pss[:].rearrange("p (gb j) -> p gb j", j=32),
                    op=AT.subtract)
            t2 = st.tile([128, 1024], F32, tag="mx_t2")
            nc.vector.tensor_tensor(t2[:], dx[:], dx[:], op=AT.mult)
            with nc.allow_low_precision(reason="fp16 rad"):
                nc.vector.tensor_tensor(rad[:], dy[:], dy[:], op=AT.mult)
                nc.vector.tensor_tensor(rad[:], rad[:], t2[:], op=AT.add)
            if first:
                nc.vector.tensor_copy(ea[:], rad[:])

        radial_part(first=True)

        # per-layer edge unit emitters ---------------------------------------
        # edge stage 1 off the PE: A = Wi.h, B = Wj.h are per-node; per-edge
        # pre-act = A_i + B_j (GpSimd fp16 broadcast add) + wr*rad + we*ea
        # (4-row Wsc matmul) summed by one DVE op.
        def emit_tAB(l, i):
            gb_l, gmp, u = i >> 2, (i >> 1) & 1, i & 1
            nb = gb_l * 128 + gmp * 64 + u * 32
            tAB = eT.tile([128, 1024], F16, tag="tAB")
            nc.gpsimd.tensor_tensor(
                tAB[:].rearrange("p (i j) -> p i j", j=32),
                A8[:, nb:nb + 32].unsqueeze(2).broadcast_to([128, 32, 32]),
                B8[:, nb:nb + 32].unsqueeze(1).broadcast_to([128, 32, 32]),
                op=AT.add)
            return tAB

        rsc_q = {}

        def emit_rsc(pair):
            """Prefetch the radial/edge_attr rows for unit pair `pair`."""
            i = pair * 2
            gb_l, gmp = i >> 2, (i >> 1) & 1
            p0 = gmp * 64
            rsc = eR.tile([4, 2048], F16, tag="rsc")
            for (row, src) in ((0, rad), (1, ea)):
                nc.sync.dma_start(
                    rsc[row:row + 1, :].rearrange(
                        "p (a b c) -> p a b c", a=2, b=32, c=32),
                    src[p0:p0 + 64, gb_l * 32:(gb_l + 1) * 32])
                nc.sync.dma_start(
                    rsc[row + 2:row + 3, :].rearrange(
                        "p (a b c) -> p a b c", a=2, b=32, c=32),
                    src[p0:p0 + 64, (gb_l + 16) * 32:(gb_l + 17) * 32])
            rsc_q[pair] = rsc

        def emit_S1(l, h, i, tAB):
            gb_l, gmp, u = i >> 2, (i >> 1) & 1, i & 1
            rsc = rsc_q[i >> 1]
            if u == 1:
                del rsc_q[i >> 1]
            ps1 = P1.tile([128, 1024], F32, tag="stage")
            for k in range(2):
                ksl = slice(u * 1024 + k * 512, u * 1024 + (k + 1) * 512)
                nc.tensor.matmul(ps1[:, k * 512:(k + 1) * 512],
                                 W(f"Wsc{l}")[0:4, :], rsc[:, ksl],
                                 start=True, stop=True)
            pre1 = eT.tile([128, 1024], F16, tag="pre1")
            with nc.allow_low_precision(reason="fp16 pre1"):
                nc.vector.tensor_tensor(pre1[:], tAB[:], ps1[:], op=AT.add)
            return pre1

        def emit_A1(l, st_u):
            m1s = eA.tile([128, 1024], F16, tag="m1s")
            nc.scalar.activation(m1s[:], st_u["pre1"][:], ACTF.Silu,
                                 bias=Bia(f"be1{l}"))
            st_u["m1s"] = m1s

        def emit_S2(l, st_u):
            ps2 = P2.tile([128, 1024], F32, tag="stage")
            m1s = st_u["m1s"]
            for k in range(2):
                ksl = slice(k * 512, (k + 1) * 512)
                nc.tensor.matmul(ps2[:, ksl], W(f"We2{l}"), m1s[:, ksl],
                                 start=True, stop=True)
            st_u["ps2"] = ps2

        def emit_A2(l, st_u):
            m_u = eM.tile([128, 1024], F16, tag="m_u")
            nc.scalar.activation(m_u[:], st_u["ps2"][:], ACTF.Silu,
                                 bias=Bia(f"be2{l}"))
            st_u["m_u"] = m_u

        def emit_red(l, st_u):
            nb = st_u["nb"]
            m_u = st_u["m_u"]
            with nc.allow_low_precision(reason="fp16 magg"):
                nc.vector.tensor_reduce(
                    magg[:, nb:nb + 32],
                    m_u[:].rearrange("p (i j) -> p i j", j=32),
                    axis=mybir.AxisListType.X, op=AT.add)
            nc.gpsimd.tensor_copy(mdiag[:, nb:nb + 32], m_u[:, 0:1024:33])

        def emit_S3(l, st_u):
            ps3 = P3.tile([128, 1024], F32, tag="stage")
            m_u = st_u["m_u"]
            for k in range(2):
                ksl = slice(k * 512, (k + 1) * 512)
                nc.tensor.matmul(ps3[:, ksl], W(f"Wc1{l}"), m_u[:, ksl],
                                 start=True, stop=True)
            st_u["ps3"] = ps3

        def emit_sq(l, st_u):
            c1t = eQ.tile([128, 1024], F16, tag="c1t")
            nc.scalar.activation(c1t[:], st_u["ps3"][:], ACTF.Silu,
                                 bias=Bia(f"bc1{l}"))
            st_u["c1t"] = c1t

        def emit_S4(l, st_u):
            g = st_u["i"] & 3
            if g == 0:
                emit_S4.ps4 = P4.tile([128, 1024], F32, tag="stage")
            ps4 = emit_S4.ps4
            c1t = st_u["c1t"]
            wsl = W(f"Wc2{l}")[:, g * 8:(g + 1) * 8]
            for k in range(2):
                ksl = slice(k * 512, (k + 1) * 512)
                nc.tensor.matmul(ps4[0:8, ksl], wsl, c1t[:, ksl],
                                 start=(g == 0), stop=(g == 3),
                                 skip_group_check=True)
            st_u["ps4"] = ps4

        def emit_out(l, st_u):
            # called on the quad's last unit: ps4 partitions 0..7 hold t for
            # 4 units x 2 halves. One extract (+bc2) and two DMAs per quad.
            i = st_u["i"]
            gb_l = i >> 2
            ps4 = st_u["ps4"]
            ssb = eS.tile([8, 1024], F32, tag="ssb")
            bc2ap = bsb[0:8, bidx[f"bc2{l}"]:bidx[f"bc2{l}"] + 1]
            nc.vector.tensor_scalar_add(ssb[:], ps4[0:8, :], bc2ap)
            nc.sync.dma_start(
                smat[:, gb_l * 32:(gb_l + 1) * 32],
                ssb[0:8:2, :].rearrange("p (i j) -> p i j", j=32))
            nc.sync.dma_start(
                smat[:, (gb_l + 16) * 32:(gb_l + 17) * 32],
                ssb[1:8:2, :].rearrange("p (i j) -> p i j", j=32))

        def emit_hupd(l, h, h_next, u):
            sl = slice(u * 1024, (u + 1) * 1024)
            psh = P3.tile([128, 1024], F32, tag="stage")
            for k in range(2):
                ksl = slice(u * 1024 + k * 512, u * 1024 + (k + 1) * 512)
                osl = slice(k * 512, (k + 1) * 512)
                nc.tensor.matmul(psh[:, osl], W(f"Wn1t{l}"), h[:, ksl],
                                 start=True, stop=False)
                nc.tensor.matmul(psh[:, osl], W(f"Wn1b{l}"), magg[:, ksl],
                                 start=False, stop=False)
                nc.tensor.matmul(psh[:, osl], W(f"Wn1d{l}"), mdiag[:, ksl],
                                 start=False, stop=True)
            hn1 = eA.tile([128, 1024], F16, tag="hn1")
            nc.scalar.activation(hn1[:], psh[:], ACTF.Silu,
                                 bias=Bia(f"bn1{l}"))
            # P3 (not P4): P4 generations now span 4 pipeline steps
            psh2 = P3.tile([128, 1024], F32, tag="stage")
            for k in range(2):
                osl = slice(k * 512, (k + 1) * 512)
                nc.tensor.matmul(psh2[:, osl], W(f"Wn2{l}"), hn1[:, osl],
                                 start=True, stop=True)
            with nc.allow_low_precision(reason="fp16 h"):
                nc.vector.scalar_tensor_tensor(
                    h_next[:, sl], psh2[:], Bia(f"bn2{l}"), h[:, sl],
                    op0=AT.add, op1=AT.add)

        def node_AB(l, h):
            """Per-node A = Wi.h, B = Wj.h and phi -> phirep for layer l."""
            for (wn, dst) in ((f"Wi{l}", A8), (f"Wj{l}", B8)):
                for u in range(NNODE // 1024):
                    psab = P1.tile([128, 1024], F32, tag="stage")
                    for k in range(2):
                        ksl = slice(u * 1024 + k * 512,
                                    u * 1024 + (k + 1) * 512)
                        nc.tensor.matmul(psab[:, k * 512:(k + 1) * 512],
                                         W(wn), h[:, ksl],
                                         start=True, stop=True)
                    with nc.allow_low_precision(reason="fp16 AB"):
                        nc.vector.tensor_copy(
                            dst[:, u * 1024:(u + 1) * 1024], psab[:])
            for u in range(NNODE // 1024):
                sl = slice(u * 1024, (u + 1) * 1024)
                psv = P1.tile([128, 1024], F32, tag="stage")
                for k in range(2):
                    ksl = slice(u * 1024 + k * 512, u * 1024 + (k + 1) * 512)
                    nc.tensor.matmul(psv[:, k * 512:(k + 1) * 512],
                                     W(f"Wv1{l}"), h[:, ksl],
                                     start=True, stop=True)
                nc.scalar.activation(hv1[:], psv[:], ACTF.Silu,
                                     bias=Bia(f"bv1{l}"))
                psv2 = P2.tile([128, 1024], F32, tag="stage")
                for k in range(2):
                    nc.tensor.matmul(psv2[:, k * 512:(k + 1) * 512],
                                     W(f"Wv2{l}"),
                                     hv1[:, k * 512:(k + 1) * 512],
                                     start=True, stop=True)
                nc.vector.tensor_scalar_add(phirep[:, sl], psv2[:],
                                            Bia(f"bv2{l}"))

        def phi_extract():
            # batch 8 transposed blocks per PSUM tile, then 2 strided copies
            for g in range(NGBL // 8):
                pst = P3.tile([128, 1024], F32, tag="stage")
                for b in range(8):
                    c = g * 8 + b
                    nc.tensor.transpose(pst[:, b * 128:(b + 1) * 128],
                                        phirep[:, c * 128:(c + 1) * 128],
                                        ident)
                nc.vector.tensor_copy(phiP[:, g * 8:(g + 1) * 8],
                                      pst[:, 0:1024:128])
                nc.vector.tensor_copy(phiP[:, NGBL + g * 8:NGBL + (g + 1) * 8],
                                      pst[:, 64:1024:128])

        def tm_chain():
            nc.scalar.activation(tm[:], rad[:], ACTF.Sqrt)
            nc.vector.tensor_scalar_add(tm[:], tm[:], 1.0)
            nc.vector.reciprocal(tm[:], tm[:])

        for l in range(L):
            h = hA if l % 2 == 0 else hB
            h_next = hB if l % 2 == 0 else hA

            if l == 0:
                tm_chain()
                node_AB(0, h)
                phi_extract()

            # ---- software-pipelined edge units ----
            NU = 64
            stq = {}

            # deep pipeline: S2 two steps behind S1, S3 four, S4 six, so
            # every stage's input is >=2 steps old when the PE reaches it
            D2, D3, D4 = 2, 4, 6

            def pipe_step(i):
                # rsc DMAs prefetched 4 units ahead so S1 never waits on them
                if i == 0:
                    for p in (0, 1, 2):
                        emit_rsc(p)
                j = i + 5
                if j < NU and (j & 1) == 0:
                    emit_rsc(j >> 1)
                if i == 0:
                    stq[0] = {"i": 0, "nb": 0, "tAB": emit_tAB(l, 0)}
                if i + 1 < NU:
                    stq[i + 1] = {"i": i + 1,
                                  "nb": ((i + 1) >> 2) * 128
                                  + ((i + 1) & 3) * 32,
                                  "tAB": emit_tAB(l, i + 1)}
                # activation/reduce consumers FIRST (their PSUM inputs were
                # written in prior steps, so each engine's FIFO head is
                # ready work and PE outputs of THIS step gate nothing ahead
                # of it), then the PE stages.
                # ACT queue in earliest-consumer-deadline order: A1 feeds S2
                # (distance 2), A2 feeds S3 (4), c1 feeds S4 (6)
                if 0 <= i - 1 < NU:
                    emit_A1(l, stq[i - 1])
                if 0 <= i - D2 - 1 < NU:
                    emit_A2(l, stq[i - D2 - 1])
                if 0 <= i - D3 - 1 < NU:
                    emit_sq(l, stq[i - D3 - 1])
                if i < NU:
                    stq[i]["pre1"] = emit_S1(l, h, i, stq[i]["tAB"])
                if 0 <= i - D2 - 1 < NU:
                    emit_red(l, stq[i - D2 - 1])
                if 0 <= i - D2 < NU:
                    emit_S2(l, stq[i - D2])
                if 0 <= i - D4 < NU:
                    emit_S4(l, stq[i - D4])
                    if ((i - D4) & 3) == 3:
                        emit_out(l, stq[i - D4])
                if 0 <= i - D3 < NU:
                    emit_S3(l, stq[i - D3])
                if 0 <= i - D4 < NU:
                    del stq[i - D4]

            for i in range(NU + D4 + 1):
                pipe_step(i)
                if i == NU // 2 + D3 + 1:
                    emit_hupd(l, h, h_next, 0)
            emit_hupd(l, h, h_next, 1)
            # next layer's node-phase PE work overlaps this layer's
            # (vector/scalar-only) matrix phase
            if l < L - 1:
                node_AB(l + 1, h_next)

            # ---- matrix phase: um, agg, vel/loc update; then radial(l+1) --
            um = st.tile([128, 1024], F32, tag="mx_um")
            nc.vector.tensor_tensor(um[:], smat[:], tm[:], op=AT.mult)
            for (dT, agg_out) in ((dx, "ax"), (dy, "ay")):
                w_ = st.tile([128, 1024], F32, tag="mx_w2")
                nc.vector.tensor_tensor(w_[:], um[:], dT[:], op=AT.mult)
                ag = st.tile([128, NGB], F32, tag="mx_" + agg_out)
                nc.vector.tensor_reduce(
                    ag[:], w_[:].rearrange("p (gb j) -> p gb j", j=32),
                    axis=mybir.AxisListType.X, op=AT.add)
                vP = velx if agg_out == "ax" else vely
                tmp = st.tile([128, NGB], F32, tag="mx_tmp")
                nc.vector.tensor_tensor(tmp[:], phiP[:], vP[:], op=AT.mult)
                nc.vector.scalar_tensor_tensor(vP[:], ag[:], 1.0 / DEG,
                                               tmp[:], op0=AT.mult,
                                               op1=AT.add)
            nc.vector.tensor_tensor(locx[:], locx[:], velx[:], op=AT.add)
            nc.vector.tensor_tensor(locy[:], locy[:], vely[:], op=AT.add)
            if l < L - 1:
                radial_part(first=False)
                tm_chain()
                phi_extract()

        # ---- output: outP interleaved (gb, c) ----
        ov = outP[:].rearrange("p (gb c) -> p gb c", c=2)
        nc.vector.tensor_scalar(ov[:, :, 0], velx[:], scale0, mean0,
                                op0=AT.mult, op1=AT.add)
        nc.vector.tensor_scalar(ov[:, :, 1], vely[:], scale1, mean1,
                                op0=AT.mult, op1=AT.add)
        nc.sync.dma_start(out_d.ap(), outP[:])

    nc.compile()
    return nc


# ----------------------------------------------------------------------------
# Entry point
# ----------------------------------------------------------------------------

def kernel(**inputs):
    import concourse.mybir  # noqa: F401  (ensure env importable)
    from concourse.bass_utils import run_bass_kernel_spmd

    inp = {k: np.asarray(v) for k, v in inputs.items()}
    obs = inp["obs"].astype(np.float32)
    scale = np.asarray(inp["scale"], np.float32)
    mean = np.asarray(inp["mean"], np.float32)

    key = (float(scale[0]), float(scale[1]), float(mean[0]), float(mean[1]))
    if key not in _BUILD_CACHE:
        _BUILD_CACHE[key] = build(*key)
    nc = _BUILD_CACHE[key]

    wpack, wpackb, fpack, biaspack = _pack_weights(inp)
    in_maps = []
    for c in range(NCORES):
        invT, locvel = _arrange_inputs(obs[c * NODES_CORE:(c + 1) * NODES_CORE])
        in_maps.append({"invT": invT, "locvel": locvel, "wpack": wpack,
                       "wpackb": wpackb, "fpack": fpack,
                        "biaspack": biaspack})
    res = run_bass_kernel_spmd(nc, in_maps, list(range(NCORES)))
    outs = [_unarrange_output(res.results[c]["out"]) for c in range(NCORES)]
    return np.concatenate(outs, axis=0)
